# revision 20
# baseline (speedup 1.0000x reference)
"""Trainium2 Bass kernel for nn_CtcBoundaryLossV3.

Reference computation (per sample b, T=2048 frames, V=1024 vocab, U=256):
  blank = ctc_log_probs[b, :, 0]
  spike[t] = (blank[t] < log(0.3)) & mask[t]
  pos = sorted spike positions; seg_j = sum(alpha[pos_j .. pos_{j+1}]) (both
  ends inclusive); boundary_j = seg_j for j < n_spikes-1, padded with 0
  loss = sum_b [ sum_{2 <= rank <= lim_b} |w(rank)-1| + relu(lim_b-1 -
         relu(nsp_b-1)) ] / B,   lim_b = min(text_len_b, 256) + 1
  where w at a spike t is the alpha-interval sum ending at t.

Segmented-scan reformulation (validated vs the jax reference):
  a0[t] = 1 - spike[t-1]
  v[t] = a0[t] * v[t-1] + alpha[t-1]          (v = w - alpha)
  w[t]-1 = vloc[t] + ploc[t]*S_excl(block) + alpha[t]-1
with vloc/ploc the per-block (64-wide) free-dim scans of the recurrence and
S_excl the cross-block affine carry.

Device layout (per core, 2 samples): [64, 64] tiles; partition p = s*32 + q,
column c, t = q*64 + c. The kernel is a single in-order DVE chain of FOUR
ops on one [64, 256] tile M = {vloc | ploc | pad | nspike}:
  1. nspike = (blank >= thr)        (spike stored inverted; thr carries the
     mask: -1e30 where masked out, so masked frames are never spikes)
  2. boundary col M[:,191] = (bcol >= thrprev)   (nspike at t = q*64-1,
     from a 63-descriptor gather of blank at block boundaries; partition
     q=0 is forced to 1 — no spike before the sample start — via a zeroed
     bcol row and thrprev = -1e30)
  3. vloc scan over a0 = M[:, 191:255] (boundary col + nspike cols 0:62)
  4. ploc scan (same a0, product form)
The block-boundary gather replaces any cross-partition work: no PE, no
PSUM, no Activation, no transposes.

DMA plan: the strided blank gather (4096 4B descriptors, the long pole on
the shared DMA engines) goes first; thr+thrprev, the boundary gather, and
alpha[t-1] (host-packed) follow inside its shadow — 4 input DMAs, and ONE
output DMA shipping M right after the last scan. The host unshard step
runs the 32-step cross-block recurrence per sample on the shipped block
summaries, composes w, applies the rank-window gate, abs, and the final
all-reduce, as the sharding hint allows.

Sharding: pure data parallel, B=16 over 8 cores (2 samples/core).
"""
import math
from contextlib import ExitStack

import numpy as np

import concourse.bacc as bacc
import concourse.tile as tile
from concourse import mybir
from concourse.bass_utils import run_bass_kernel_spmd

f32 = mybir.dt.float32
Alu = mybir.AluOpType

N_CORES = 8
B_FULL, T, V, U = 16, 2048, 1024, 256
B_LOC = B_FULL // N_CORES  # 2 samples per core
NBK = 32   # blocks per sample
BC = 64    # columns (t) per block
P = 64     # partitions = 2 samples * NBK
LOG_THRESH = math.log(1.0 - 0.7)  # log(0.3); compared in f32 on device

S_VLOC = 0 * BC    # vloc
S_PLOC = 1 * BC    # ploc
S_PAD = 2 * BC     # junk slot; its last column (191) is the a0 boundary col
S_NSPK = 3 * BC    # nspike


def _body(ctx, tc, ctc_d, thr_d, aprev_d, out_d):
    nc = tc.nc
    pool = ctx.enter_context(tc.tile_pool(name="p", bufs=1))

    blank = pool.tile([P, BC], f32)
    thr = pool.tile([P, BC + 1], f32)
    aprev = pool.tile([P, BC], f32)
    bcol = pool.tile([P, 1], f32)
    M = pool.tile([P, 4 * BC], f32)

    # DVE zeroing during the DMA window: bcol row 0 only (no frame before
    # the very first block; rows 1: are DMA-written, so no overlap and no
    # ordering dependency) and the pad slot (shipped, ignored by the host).
    nc.vector.memset(bcol[0:1], 0.0)
    nc.vector.memset(M[:, S_PAD : S_PAD + BC], 0.0)

    # ---- input DMAs. The strided blank gather first on the SP ring (it is
    # the long pole on the shared DMA engines); the boundary gather and the
    # two small host-packed tensors ride behind it in its shadow.
    blank_r = ctc_d[:, :, 0].rearrange("s (q c) -> (s q) c", c=BC)
    nc.sync.dma_start(out=blank[:], in_=blank_r)
    nc.scalar.dma_start(out=thr[:], in_=thr_d[:])
    # blank at t = p*64 - 1 (p = global block index 1..63; flat across the
    # two samples, so p=32 reads sample 0's frame 2047 — a don't-care row,
    # like p=0, forced to "no spike" by thrprev = -1e30)
    bcol_r = ctc_d.rearrange("s t v -> (s t) v")[BC - 1 : B_LOC * T - 1 : BC, 0:1]
    nc.sync.dma_start(out=bcol[1:P], in_=bcol_r)
    nc.scalar.dma_start(out=aprev[:], in_=aprev_d[:])

    # ---- single DVE chain (4 ops) ----
    nc.vector.tensor_tensor(out=M[:, S_NSPK : S_NSPK + BC], in0=blank[:],
                            in1=thr[:, 0:BC], op=Alu.is_ge)
    nc.vector.tensor_tensor(out=M[:, S_NSPK - 1 : S_NSPK], in0=bcol[:],
                            in1=thr[:, BC : BC + 1], op=Alu.is_ge)
    a0 = M[:, S_NSPK - 1 : S_NSPK + BC - 1]
    # v[c] = a0[c]*v[c-1] + alpha[t-1]; ploc[c] = prod a0[0..c]
    nc.vector.tensor_tensor_scan(out=M[:, S_VLOC : S_VLOC + BC], data0=a0,
                                 data1=aprev[:], initial=0.0,
                                 op0=Alu.mult, op1=Alu.add)
    nc.vector.tensor_tensor_scan(out=M[:, S_PLOC : S_PLOC + BC], data0=a0,
                                 data1=a0, initial=1.0,
                                 op0=Alu.mult, op1=Alu.bypass)
    # one output DMA: {vloc | ploc | pad | nspike}
    nc.sync.dma_start(out=out_d[:], in_=M[:])


def build_nc():
    nc = bacc.Bacc("TRN2", target_bir_lowering=False, debug=False,
                   num_devices=N_CORES)
    ctc_d = nc.dram_tensor("ctc", [B_LOC, T, V], f32, kind="ExternalInput")
    thr_d = nc.dram_tensor("thr", [P, BC + 1], f32, kind="ExternalInput")
    aprev_d = nc.dram_tensor("aprev", [P, BC], f32, kind="ExternalInput")
    out_d = nc.dram_tensor("out", [P, 4 * BC], f32, kind="ExternalOutput")
    with tile.TileContext(nc) as tc:
        with ExitStack() as ctx:
            _body(ctx, tc, ctc_d.ap(), thr_d.ap(), aprev_d.ap(), out_d.ap())
    nc.compile()
    return nc


_NC_CACHE = None


def _get_nc():
    global _NC_CACHE
    if _NC_CACHE is None:
        _NC_CACHE = build_nc()
    return _NC_CACHE


def make_in_maps(alpha, ctc_log_probs, mask, text_length):
    in_maps = []
    for i in range(N_CORES):
        sl = slice(i * B_LOC, (i + 1) * B_LOC)
        a = np.asarray(alpha[sl], np.float32)
        m = np.asarray(mask[sl], bool)
        # thr cols 0:64: per-frame threshold (mask folded in). col 64: the
        # threshold for the block-boundary frame t = q*64-1; -1e30 for q=0
        # (and the cross-sample don't-care row q=32) forces nspike=1 there.
        thr = np.full((P, BC + 1), np.float32(-1e30), np.float32)
        thr[:, 0:BC] = np.where(m, np.float32(LOG_THRESH),
                                np.float32(-1e30)).reshape(P, BC)
        mprev = m.reshape(P, BC)[:, BC - 1]  # mask at t = q*64+63
        thr[1:P, BC] = np.where(mprev[0 : P - 1], np.float32(LOG_THRESH),
                                np.float32(-1e30))
        thr[NBK, BC] = np.float32(-1e30)  # sample-1 block 0: no prev frame
        aprev = np.zeros((B_LOC, T), np.float32)
        aprev[:, 1:] = a[:, :-1]
        in_maps.append(
            {
                "ctc": np.ascontiguousarray(ctc_log_probs[sl]),
                "thr": np.ascontiguousarray(thr),
                "aprev": np.ascontiguousarray(aprev.reshape(P, BC)),
            }
        )
    return in_maps


def postprocess(res, alpha, text_length):
    """Host unshard + final reduction: run the cross-block recurrence on
    the shipped block summaries, compose w, gate by the rank window, abs,
    sum, /B."""
    alpha = np.asarray(alpha, np.float32)
    text_length = np.asarray(text_length, np.int64)
    total = np.float32(0.0)
    for i, r in enumerate(res.results):
        out = r["out"].astype(np.float32)            # [64, 256]
        for s in range(B_LOC):
            b = i * B_LOC + s
            rows = slice(s * NBK, (s + 1) * NBK)
            vloc = out[rows, S_VLOC : S_VLOC + BC]
            ploc = out[rows, S_PLOC : S_PLOC + BC]
            spike = 1.0 - out[rows, S_NSPK : S_NSPK + BC].reshape(T)
            # cross-block affine recurrence on block summaries (exclusive)
            sexcl = np.zeros(NBK, np.float32)
            st = np.float32(0.0)
            for q in range(NBK):
                sexcl[q] = st
                st = ploc[q, BC - 1] * st + vloc[q, BC - 1]
            w0 = vloc + ploc * sexcl[:, None]
            wm1 = w0.reshape(T) + alpha[b] - np.float32(1.0)
            rank = np.cumsum(spike, dtype=np.float32)
            lim = np.float32(min(int(text_length[b]), min(T - 1, U)) + 1)
            gate = (spike > 0.5) & (rank >= 2.0) & (rank <= lim)
            part = np.abs(wm1[gate]).sum(dtype=np.float32)
            nsp = rank[-1] if T else np.float32(0.0)
            corr = max(lim - 1.0 - max(nsp - 1.0, 0.0), 0.0)
            total += part + np.float32(corr)
    return np.asarray(total / np.float32(B_FULL), dtype=np.float32)


def kernel(alpha, ctc_log_probs, mask, text_length):
    nc = _get_nc()
    in_maps = make_in_maps(alpha, ctc_log_probs, mask, text_length)
    res = run_bass_kernel_spmd(nc, in_maps, list(range(N_CORES)))
    return postprocess(res, alpha, text_length)


# revision 25
# speedup vs baseline: 1.0054x; 1.0054x over previous
"""Trainium2 Bass kernel for nn_CtcBoundaryLossV3.

Reference computation (per sample b, T=2048 frames, V=1024 vocab, U=256):
  blank = ctc_log_probs[b, :, 0]
  spike[t] = (blank[t] < log(0.3)) & mask[t]
  pos = sorted spike positions; seg_j = sum(alpha[pos_j .. pos_{j+1}]) (both
  ends inclusive); boundary_j = seg_j for j < n_spikes-1, padded with 0
  loss = sum_b [ sum_{2 <= rank <= lim_b} |w(rank)-1| + relu(lim_b-1 -
         relu(nsp_b-1)) ] / B,   lim_b = min(text_len_b, 256) + 1
  where w at a spike t is the alpha-interval sum ending at t.

Segmented-scan reformulation (validated vs the jax reference):
  a0[t] = 1 - spike[t-1]
  v[t] = a0[t] * v[t-1] + alpha[t-1]          (v = w - alpha)
  w[t]-1 = vloc[t] + ploc[t]*S_excl(block) + alpha[t]-1
with vloc/ploc the per-block (64-wide) free-dim scans of the recurrence and
S_excl the cross-block affine carry.

Device layout (per core, 2 samples): [64, 64] tiles; partition p = s*32 + q,
column c, t = q*64 + c. The kernel is a single in-order DVE chain of FOUR
ops on one [64, 256] tile M = {vloc | ploc | pad | nspike}:
  1. nspike = (blank >= thr)        (spike stored inverted; thr carries the
     mask: -1e30 where masked out, so masked frames are never spikes)
  2. boundary col M[:,191] = (bcol >= thrprev)   (nspike at t = q*64-1,
     from a 63-descriptor gather of blank at block boundaries; partition
     q=0 is forced to 1 — no spike before the sample start — via a zeroed
     bcol row and thrprev = -1e30)
  3. vloc scan over a0 = M[:, 191:255] (boundary col + nspike cols 0:62)
  4. ploc scan (same a0, product form)
The block-boundary gather replaces any cross-partition work: no PE, no
PSUM, no Activation, no transposes.

DMA plan: the strided blank gather (4096 4B descriptors, the long pole on
the shared DMA engines) goes first; thr+thrprev, the boundary gather, and
alpha[t-1] (host-packed) follow inside its shadow — 4 input DMAs, and ONE
output DMA shipping M right after the last scan. The host unshard step
runs the 32-step cross-block recurrence per sample on the shipped block
summaries, composes w, applies the rank-window gate, abs, and the final
all-reduce, as the sharding hint allows.

Sharding: pure data parallel, B=16 over 8 cores (2 samples/core).
"""
import math
from contextlib import ExitStack

import numpy as np

import concourse.bacc as bacc
import concourse.tile as tile
from concourse import mybir
from concourse.bass_utils import run_bass_kernel_spmd

f32 = mybir.dt.float32
Alu = mybir.AluOpType

N_CORES = 8
B_FULL, T, V, U = 16, 2048, 1024, 256
B_LOC = B_FULL // N_CORES  # 2 samples per core
NBK = 32   # blocks per sample
BC = 64    # columns (t) per block
P = 64     # partitions = 2 samples * NBK
LOG_THRESH = math.log(1.0 - 0.7)  # log(0.3); compared in f32 on device

S_PAD = 0 * BC     # junk slot (not shipped); col 63 is the a0 boundary col
S_NSPK = 1 * BC    # nspike
S_VLOC = 2 * BC    # vloc
S_PLOC = 3 * BC    # ploc


def _body(ctx, tc, ctc_d, thr_d, aprev_d, out_d):
    nc = tc.nc
    pool = ctx.enter_context(tc.tile_pool(name="p", bufs=1))

    blank = pool.tile([P, BC], f32)
    thr = pool.tile([P, BC + 1], f32)
    aprev = pool.tile([P, BC], f32)
    bcol = pool.tile([P, 1], f32)
    M = pool.tile([P, 4 * BC], f32)

    # DVE zeroing during the DMA window: bcol row 0 only (no frame before
    # the very first block; rows 1: are DMA-written, so no overlap and no
    # ordering dependency).
    nc.vector.memset(bcol[0:1], 0.0)

    # ---- input DMAs. The strided blank gather first on the SP ring (it is
    # the long pole on the shared DMA engines); the boundary gather and the
    # two small host-packed tensors ride behind it in its shadow. aprev
    # goes third: it gates the vloc scan, while the boundary compare (bcol,
    # fourth) is a zero-duration op off the chain's critical path.
    blank_r = ctc_d[:, :, 0].rearrange("s (q c) -> (s q) c", c=BC)
    nc.sync.dma_start(out=blank[:], in_=blank_r)
    nc.scalar.dma_start(out=thr[:], in_=thr_d[:])
    nc.scalar.dma_start(out=aprev[:], in_=aprev_d[:])
    # blank at t = p*64 - 1 (p = global block index 1..63; flat across the
    # two samples, so p=32 reads sample 0's frame 2047 — a don't-care row,
    # like p=0, forced to "no spike" by thrprev = -1e30)
    bcol_r = ctc_d.rearrange("s t v -> (s t) v")[BC - 1 : B_LOC * T - 1 : BC, 0:1]
    nc.sync.dma_start(out=bcol[1:P], in_=bcol_r)

    # ---- single DVE chain (4 ops) ----
    nc.vector.tensor_tensor(out=M[:, S_NSPK : S_NSPK + BC], in0=blank[:],
                            in1=thr[:, 0:BC], op=Alu.is_ge)
    nc.vector.tensor_tensor(out=M[:, S_NSPK - 1 : S_NSPK], in0=bcol[:],
                            in1=thr[:, BC : BC + 1], op=Alu.is_ge)
    a0 = M[:, S_NSPK - 1 : S_NSPK + BC - 1]
    # v[c] = a0[c]*v[c-1] + alpha[t-1]; ploc[c] = prod a0[0..c]
    nc.vector.tensor_tensor_scan(out=M[:, S_VLOC : S_VLOC + BC], data0=a0,
                                 data1=aprev[:], initial=0.0,
                                 op0=Alu.mult, op1=Alu.add)
    nc.vector.tensor_tensor_scan(out=M[:, S_PLOC : S_PLOC + BC], data0=a0,
                                 data1=a0, initial=1.0,
                                 op0=Alu.mult, op1=Alu.bypass)
    # one output DMA: {nspike | vloc | ploc} (the pad slot stays on device)
    nc.sync.dma_start(out=out_d[:], in_=M[:, S_NSPK : S_PLOC + BC])


def build_nc():
    nc = bacc.Bacc("TRN2", target_bir_lowering=False, debug=False,
                   num_devices=N_CORES)
    ctc_d = nc.dram_tensor("ctc", [B_LOC, T, V], f32, kind="ExternalInput")
    thr_d = nc.dram_tensor("thr", [P, BC + 1], f32, kind="ExternalInput")
    aprev_d = nc.dram_tensor("aprev", [P, BC], f32, kind="ExternalInput")
    out_d = nc.dram_tensor("out", [P, 3 * BC], f32, kind="ExternalOutput")
    with tile.TileContext(nc) as tc:
        with ExitStack() as ctx:
            _body(ctx, tc, ctc_d.ap(), thr_d.ap(), aprev_d.ap(), out_d.ap())
    nc.compile()
    return nc


_NC_CACHE = None


def _get_nc():
    global _NC_CACHE
    if _NC_CACHE is None:
        _NC_CACHE = build_nc()
    return _NC_CACHE


def make_in_maps(alpha, ctc_log_probs, mask, text_length):
    in_maps = []
    for i in range(N_CORES):
        sl = slice(i * B_LOC, (i + 1) * B_LOC)
        a = np.asarray(alpha[sl], np.float32)
        m = np.asarray(mask[sl], bool)
        # thr cols 0:64: per-frame threshold (mask folded in). col 64: the
        # threshold for the block-boundary frame t = q*64-1; -1e30 for q=0
        # (and the cross-sample don't-care row q=32) forces nspike=1 there.
        thr = np.full((P, BC + 1), np.float32(-1e30), np.float32)
        thr[:, 0:BC] = np.where(m, np.float32(LOG_THRESH),
                                np.float32(-1e30)).reshape(P, BC)
        mprev = m.reshape(P, BC)[:, BC - 1]  # mask at t = q*64+63
        thr[1:P, BC] = np.where(mprev[0 : P - 1], np.float32(LOG_THRESH),
                                np.float32(-1e30))
        thr[NBK, BC] = np.float32(-1e30)  # sample-1 block 0: no prev frame
        aprev = np.zeros((B_LOC, T), np.float32)
        aprev[:, 1:] = a[:, :-1]
        in_maps.append(
            {
                "ctc": np.ascontiguousarray(ctc_log_probs[sl]),
                "thr": np.ascontiguousarray(thr),
                "aprev": np.ascontiguousarray(aprev.reshape(P, BC)),
            }
        )
    return in_maps


def postprocess(res, alpha, text_length):
    """Host unshard + final reduction: run the cross-block recurrence on
    the shipped block summaries, compose w, gate by the rank window, abs,
    sum, /B."""
    alpha = np.asarray(alpha, np.float32)
    text_length = np.asarray(text_length, np.int64)
    total = np.float32(0.0)
    for i, r in enumerate(res.results):
        out = r["out"].astype(np.float32)            # [64, 192] nspk|vloc|ploc
        for s in range(B_LOC):
            b = i * B_LOC + s
            rows = slice(s * NBK, (s + 1) * NBK)
            spike = 1.0 - out[rows, 0:BC].reshape(T)
            vloc = out[rows, BC : 2 * BC]
            ploc = out[rows, 2 * BC : 3 * BC]
            # cross-block affine recurrence on block summaries (exclusive)
            sexcl = np.zeros(NBK, np.float32)
            st = np.float32(0.0)
            for q in range(NBK):
                sexcl[q] = st
                st = ploc[q, BC - 1] * st + vloc[q, BC - 1]
            w0 = vloc + ploc * sexcl[:, None]
            wm1 = w0.reshape(T) + alpha[b] - np.float32(1.0)
            rank = np.cumsum(spike, dtype=np.float32)
            lim = np.float32(min(int(text_length[b]), min(T - 1, U)) + 1)
            gate = (spike > 0.5) & (rank >= 2.0) & (rank <= lim)
            part = np.abs(wm1[gate]).sum(dtype=np.float32)
            nsp = rank[-1] if T else np.float32(0.0)
            corr = max(lim - 1.0 - max(nsp - 1.0, 0.0), 0.0)
            total += part + np.float32(corr)
    return np.asarray(total / np.float32(B_FULL), dtype=np.float32)


def kernel(alpha, ctc_log_probs, mask, text_length):
    nc = _get_nc()
    in_maps = make_in_maps(alpha, ctc_log_probs, mask, text_length)
    res = run_bass_kernel_spmd(nc, in_maps, list(range(N_CORES)))
    return postprocess(res, alpha, text_length)


# revision 31
# speedup vs baseline: 1.0177x; 1.0122x over previous
"""Trainium2 Bass kernel for nn_CtcBoundaryLossV3.

Reference computation (per sample b, T=2048 frames, V=1024 vocab, U=256):
  blank = ctc_log_probs[b, :, 0]
  spike[t] = (blank[t] < log(0.3)) & mask[t]
  pos = sorted spike positions; seg_j = sum(alpha[pos_j .. pos_{j+1}]) (both
  ends inclusive); boundary_j = seg_j for j < n_spikes-1, padded with 0
  loss = sum_b [ sum_{2 <= rank <= lim_b} |w(rank)-1| + relu(lim_b-1 -
         relu(nsp_b-1)) ] / B,   lim_b = min(text_len_b, 256) + 1
  where w at a spike t is the alpha-interval sum ending at t.

Segmented-scan reformulation (validated vs the jax reference):
  a0[t] = 1 - spike[t-1]
  v[t] = a0[t] * v[t-1] + alpha[t-1]          (v = w - alpha)
  w[t]-1 = vloc[t] + ploc[t]*S_excl(block) + alpha[t]-1
with vloc/ploc the per-block (64-wide) free-dim scans of the recurrence and
S_excl the cross-block affine carry.

Device layout (per core, 2 samples): [64, 64] tiles; partition p = s*32 + q,
column c, t = q*64 + c. The kernel is a single in-order DVE chain of FOUR
ops on one [64, 256] tile M = {vloc | ploc | pad | nspike}:
  1. nspike = (blank >= thr)        (spike stored inverted; thr carries the
     mask: -1e30 where masked out, so masked frames are never spikes)
  2. boundary col M[:,191] = (bcol >= thrprev)   (nspike at t = q*64-1,
     from a 63-descriptor gather of blank at block boundaries; partition
     q=0 is forced to 1 — no spike before the sample start — via a zeroed
     bcol row and thrprev = -1e30)
  3. vloc scan over a0 = M[:, 191:255] (boundary col + nspike cols 0:62)
  4. ploc scan (same a0, product form)
The block-boundary gather replaces any cross-partition work: no PE, no
PSUM, no Activation, no transposes.

DMA plan: the strided blank gather (4096 4B descriptors, the long pole on
the shared DMA engines) goes first; thr+thrprev, the boundary gather, and
alpha[t-1] (host-packed) follow inside its shadow — 4 input DMAs, and ONE
output DMA shipping M right after the last scan. The host unshard step
runs the 32-step cross-block recurrence per sample on the shipped block
summaries, composes w, applies the rank-window gate, abs, and the final
all-reduce, as the sharding hint allows.

Sharding: pure data parallel, B=16 over 8 cores (2 samples/core).
"""
import math
from contextlib import ExitStack

import numpy as np

import concourse.bacc as bacc
import concourse.tile as tile
from concourse import mybir
from concourse.bass_utils import run_bass_kernel_spmd

f32 = mybir.dt.float32
Alu = mybir.AluOpType

N_CORES = 8
B_FULL, T, V, U = 16, 2048, 1024, 256
B_LOC = B_FULL // N_CORES  # 2 samples per core
NBK = 32   # blocks per sample
BC = 64    # columns (t) per block
P = 64     # partitions = 2 samples * NBK
LOG_THRESH = math.log(1.0 - 0.7)  # log(0.3); compared in f32 on device

S_PAD = 0 * BC     # junk slot (not shipped); col 63 is the a0 boundary col
S_NSPK = 1 * BC    # nspike
S_VLOC = 2 * BC    # vloc
S_PLOC = 3 * BC    # ploc


def _body(ctx, tc, ctc_d, pack_d, out_d):
    nc = tc.nc
    pool = ctx.enter_context(tc.tile_pool(name="p", bufs=1))

    blank = pool.tile([P, BC], f32)
    # pack = {thr (0:64) | thrprev (64) | aprev (65:129)} in one DMA
    pack = pool.tile([P, 2 * BC + 1], f32)
    bcol = pool.tile([P, 1], f32)
    M = pool.tile([P, 4 * BC], f32)

    # DVE zeroing during the DMA window: bcol row 0 only (no frame before
    # the very first block; rows 1: are DMA-written, so no overlap and no
    # ordering dependency).
    nc.vector.memset(bcol[0:1], 0.0)

    # ---- input DMAs (3 total). The strided blank gather first on the SP
    # ring (it is the long pole on the shared DMA engines); the host pack
    # (thresholds + alpha[t-1]) and the boundary gather in its shadow.
    blank_r = ctc_d[:, :, 0].rearrange("s (q c) -> (s q) c", c=BC)
    nc.sync.dma_start(out=blank[:], in_=blank_r)
    nc.scalar.dma_start(out=pack[:], in_=pack_d[:])
    # blank at t = p*64 - 1 (p = global block index 1..63; flat across the
    # two samples, so p=32 reads sample 0's frame 2047 — a don't-care row,
    # like p=0, forced to "no spike" by thrprev = -1e30)
    bcol_r = ctc_d.rearrange("s t v -> (s t) v")[BC - 1 : B_LOC * T - 1 : BC, 0:1]
    nc.sync.dma_start(out=bcol[1:P], in_=bcol_r)

    # ---- single DVE chain (4 ops) ----
    nc.vector.tensor_tensor(out=M[:, S_NSPK : S_NSPK + BC], in0=blank[:],
                            in1=pack[:, 0:BC], op=Alu.is_ge)
    nc.vector.tensor_tensor(out=M[:, S_NSPK - 1 : S_NSPK], in0=bcol[:],
                            in1=pack[:, BC : BC + 1], op=Alu.is_ge)
    a0 = M[:, S_NSPK - 1 : S_NSPK + BC - 1]
    # v[c] = a0[c]*v[c-1] + alpha[t-1]; ploc[c] = prod a0[0..c]
    nc.vector.tensor_tensor_scan(out=M[:, S_VLOC : S_VLOC + BC], data0=a0,
                                 data1=pack[:, BC + 1 : 2 * BC + 1],
                                 initial=0.0, op0=Alu.mult, op1=Alu.add)
    nc.vector.tensor_tensor_scan(out=M[:, S_PLOC : S_PLOC + BC], data0=a0,
                                 data1=a0, initial=1.0,
                                 op0=Alu.mult, op1=Alu.bypass)
    # one output DMA: {nspike | vloc | ploc} (the pad slot stays on device)
    nc.sync.dma_start(out=out_d[:], in_=M[:, S_NSPK : S_PLOC + BC])


def build_nc():
    nc = bacc.Bacc("TRN2", target_bir_lowering=False, debug=False,
                   num_devices=N_CORES)
    ctc_d = nc.dram_tensor("ctc", [B_LOC, T, V], f32, kind="ExternalInput")
    pack_d = nc.dram_tensor("pack", [P, 2 * BC + 1], f32, kind="ExternalInput")
    out_d = nc.dram_tensor("out", [P, 3 * BC], f32, kind="ExternalOutput")
    with tile.TileContext(nc) as tc:
        with ExitStack() as ctx:
            _body(ctx, tc, ctc_d.ap(), pack_d.ap(), out_d.ap())
    nc.compile()
    return nc


_NC_CACHE = None


def _get_nc():
    global _NC_CACHE
    if _NC_CACHE is None:
        _NC_CACHE = build_nc()
    return _NC_CACHE


def make_in_maps(alpha, ctc_log_probs, mask, text_length):
    in_maps = []
    for i in range(N_CORES):
        sl = slice(i * B_LOC, (i + 1) * B_LOC)
        a = np.asarray(alpha[sl], np.float32)
        m = np.asarray(mask[sl], bool)
        # pack cols 0:64: per-frame threshold (mask folded in). col 64: the
        # threshold for the block-boundary frame t = q*64-1; -1e30 for q=0
        # (and the cross-sample don't-care row q=32) forces nspike=1 there.
        # cols 65:129: alpha[t-1].
        pack = np.full((P, 2 * BC + 1), np.float32(-1e30), np.float32)
        pack[:, 0:BC] = np.where(m, np.float32(LOG_THRESH),
                                 np.float32(-1e30)).reshape(P, BC)
        mprev = m.reshape(P, BC)[:, BC - 1]  # mask at t = q*64+63
        pack[1:P, BC] = np.where(mprev[0 : P - 1], np.float32(LOG_THRESH),
                                 np.float32(-1e30))
        pack[NBK, BC] = np.float32(-1e30)  # sample-1 block 0: no prev frame
        aprev = np.zeros((B_LOC, T), np.float32)
        aprev[:, 1:] = a[:, :-1]
        pack[:, BC + 1 : 2 * BC + 1] = aprev.reshape(P, BC)
        in_maps.append(
            {
                "ctc": np.ascontiguousarray(ctc_log_probs[sl]),
                "pack": np.ascontiguousarray(pack),
            }
        )
    return in_maps


def postprocess(res, alpha, text_length):
    """Host unshard + final reduction: run the cross-block recurrence on
    the shipped block summaries, compose w, gate by the rank window, abs,
    sum, /B."""
    alpha = np.asarray(alpha, np.float32)
    text_length = np.asarray(text_length, np.int64)
    total = np.float32(0.0)
    for i, r in enumerate(res.results):
        out = r["out"].astype(np.float32)            # [64, 192] nspk|vloc|ploc
        for s in range(B_LOC):
            b = i * B_LOC + s
            rows = slice(s * NBK, (s + 1) * NBK)
            spike = 1.0 - out[rows, 0:BC].reshape(T)
            vloc = out[rows, BC : 2 * BC]
            ploc = out[rows, 2 * BC : 3 * BC]
            # cross-block affine recurrence on block summaries (exclusive)
            sexcl = np.zeros(NBK, np.float32)
            st = np.float32(0.0)
            for q in range(NBK):
                sexcl[q] = st
                st = ploc[q, BC - 1] * st + vloc[q, BC - 1]
            w0 = vloc + ploc * sexcl[:, None]
            wm1 = w0.reshape(T) + alpha[b] - np.float32(1.0)
            rank = np.cumsum(spike, dtype=np.float32)
            lim = np.float32(min(int(text_length[b]), min(T - 1, U)) + 1)
            gate = (spike > 0.5) & (rank >= 2.0) & (rank <= lim)
            part = np.abs(wm1[gate]).sum(dtype=np.float32)
            nsp = rank[-1] if T else np.float32(0.0)
            corr = max(lim - 1.0 - max(nsp - 1.0, 0.0), 0.0)
            total += part + np.float32(corr)
    return np.asarray(total / np.float32(B_FULL), dtype=np.float32)


def kernel(alpha, ctc_log_probs, mask, text_length):
    nc = _get_nc()
    in_maps = make_in_maps(alpha, ctc_log_probs, mask, text_length)
    res = run_bass_kernel_spmd(nc, in_maps, list(range(N_CORES)))
    return postprocess(res, alpha, text_length)
